# revision 7
# baseline (speedup 1.0000x reference)
"""8-core tensor-parallel GQA attention for TRN2 — ACT-paced schedule.

Problem: x[2,2048,1024] -> QKV proj -> 16-head attention (4 KV heads,
GQA groups of 4) -> out proj.  Sharding: 2 query heads + their shared
KV head per core; o_proj row-parallel with host-side partial-sum
reduce (host reduce is free for HW exec time).

Engine budget per core (the floor):
  ACT: 128 exp tiles [128,1024] ~1.04us each  -> ~133us  (hard floor;
       exp exists only on the Activation engine)
  PE:  ~295k cycles @2.4GHz                   -> ~124us
Everything else (DVE/Pool/DMA) is kept off those two engines.

Schedule: the kt loop is paced by ACT (2 exps/kt, 2076ns).  PE per kt
does the ST pair (row-tiled, both heads in disjoint PE row groups,
~480ns) + AV pair (~960ns), leaving ~630ns/kt of slack that absorbs
the projection / o_proj "fill" work.  PSUM (8 banks):
  stA, stB: one [128,1024] f32 tile per head (4 banks).  bufs=1 each
       makes the WAR chain ST_kt+1 <- exp_kt explicit.
  psot: ring of 2x4KB slots (4 banks) serving the AV accumulators
       [65,1024] AND the proj/o_proj fill chunks.  Both accumulators
       are allocated back-to-back so they always land in distinct
       slots.
A deep SBUF pool of P tiles (exp outputs) lets AV run several kt
behind exp, so fill bursts don't stall ACT: ST/exp continue at full
cadence while AV + fills share the PE.  The AV backlog cap tightens
near each q-tile boundary so the boundary drain stays short.

The [V|1] stationary trick accumulates the softmax denominator in
row 64 of the AV accumulator for free; normalize = recip + partition
broadcast + mul on DVE/Pool only.
"""

import os
import sys
from collections import deque

import numpy as np

for _p in ("/opt/trn_rl_repo", "/root/.axon_site/_ro/trn_rl_repo"):
    if os.path.isdir(_p) and _p not in sys.path:
        sys.path.append(_p)

import concourse.bass as bass
import concourse.tile as tile
from concourse import bacc, mybir
from concourse.bass_utils import run_bass_kernel_spmd

AF = mybir.ActivationFunctionType
F32 = mybir.dt.float32

B, N, D = 2, 2048, 1024
BN = B * N
HEADS, KV_HEADS, HD = 16, 4, 64
SCALE = HD ** -0.5
NCORES = 8
HPC = HEADS // NCORES          # query heads per core = 2
JC = HPC * HD                  # per-core head-dim columns = 128
KC = D // 128                  # contraction chunks for projections = 8
PSD = 512                      # psum bank size in f32 / matmul col cap
QW = 1024                      # attention q-tile width
KTS = N // 128                 # key tiles per batch = 16
NQT = N // QW                  # q tiles per batch = 2

MM_MODE = os.environ.get("KERNEL_MM_DTYPE", "bfloat16")
P_DEPTH = int(os.environ.get("KERNEL_P_DEPTH", "20"))     # P-tile ring
CAP_KTS = P_DEPTH // 2 - 1                                # max AV lag in kts
SLOT_FILL_NS = float(os.environ.get("KERNEL_SLOT_FILL", "1100"))
FILLER = int(os.environ.get("KERNEL_FILLER", "2"))        # dummy ldw pad

_NC_CACHE: dict[tuple, object] = {}


def _storage_dt(mode):
    if mode == "bfloat16":
        return mybir.dt.bfloat16
    if mode == "float32r":
        return mybir.dt.float32r
    return F32


def _np_dt(mode):
    if mode == "bfloat16":
        import ml_dtypes
        return ml_dtypes.bfloat16
    return np.float32


def _build_program(mode):
    sdt = _storage_dt(mode)
    nc = bacc.Bacc("TRN2", target_bir_lowering=False, debug=False)

    xT = nc.dram_tensor("xT", [D, BN], sdt, kind="ExternalInput")
    wq = nc.dram_tensor("wq", [D, JC], sdt, kind="ExternalInput")
    wkv = nc.dram_tensor("wkv", [D, JC], sdt, kind="ExternalInput")
    wo = nc.dram_tensor("wo", [JC, D], sdt, kind="ExternalInput")
    bq = nc.dram_tensor("bq", [JC, 1], F32, kind="ExternalInput")
    bkv = nc.dram_tensor("bkv", [JC, 1], F32, kind="ExternalInput")
    ident_d = nc.dram_tensor("ident", [64, 64], sdt, kind="ExternalInput")
    ones_d = nc.dram_tensor("ones", [128, KTS], sdt, kind="ExternalInput")
    out = nc.dram_tensor("out", [BN, D], F32, kind="ExternalOutput")

    xTr = xT[:].rearrange("(c p) n -> c p n", p=128)
    wqr = wq[:].rearrange("(c p) j -> c p j", p=128)
    wkvr = wkv[:].rearrange("(c p) j -> c p j", p=128)

    # q-tiles in execution order
    tiles = [(b, qt * QW) for b in range(B) for qt in range(NQT)]
    NT = len(tiles)

    with tile.TileContext(nc) as tc:
        with (
            tc.tile_pool(name="consts", bufs=1) as consts,
            tc.tile_pool(name="xin", bufs=3) as xin,
            tc.tile_pool(name="big", bufs=1) as big,
            tc.tile_pool(name="ptp", bufs=P_DEPTH) as ptp,
            tc.tile_pool(name="stat", bufs=2) as stat,
            tc.tile_pool(name="outp", bufs=4) as outp,
            tc.tile_pool(name="stp", bufs=1, space="PSUM") as stp,
            tc.tile_pool(name="psot", bufs=2, space="PSUM") as psot,
        ):
            wq_sb = consts.tile([128, KC, 128], sdt, tag="wq")
            wkv_sb = consts.tile([128, KC, 128], sdt, tag="wkv")
            wo_sb = consts.tile([128, D], sdt, tag="wo")
            bq_sb = consts.tile([128, 1], F32, tag="bq")
            bkv_sb = consts.tile([128, 1], F32, tag="bkv")
            ident = consts.tile([64, 64], sdt, tag="ident")
            # constants on the SWDGE queue; x streams on sync HWDGE
            for c in range(KC):
                nc.gpsimd.dma_start(wq_sb[:, c, :], wqr[c])
                nc.gpsimd.dma_start(wkv_sb[:, c, :], wkvr[c])
            nc.gpsimd.dma_start(wo_sb[:], wo[:])
            nc.gpsimd.dma_start(bq_sb[:], bq[:])
            nc.gpsimd.dma_start(bkv_sb[:], bkv[:])
            nc.gpsimd.dma_start(ident[:], ident_d[:])

            QT, KVT, KT2, VO, OT = {}, {}, {}, {}, {}
            for b in range(B):
                QT[b] = big.tile([128, N], sdt, tag=f"QT{b}", name=f"QT{b}")
                KVT[b] = big.tile([128, N], sdt, tag=f"KVT{b}", name=f"KVT{b}")
                KT2[b] = big.tile([128, KTS, 128], sdt, tag=f"KT2{b}",
                                  name=f"KT2{b}")
                VO[b] = big.tile([128, KTS, 65], sdt, tag=f"VO{b}", name=f"VO{b}")
                OT[b] = big.tile([128, N // 128, 128], sdt, tag=f"OT{b}",
                                 name=f"OT{b}")
                nc.gpsimd.dma_start(
                    VO[b][:, :, 64:65], ones_d[:].rearrange("p (k o) -> p k o", o=1)
                )

            # ---- x loads: one DMA per [128, KC, 1024] tile (3D AP); the
            # first tile is split in half so compute starts after ~1MB
            xts = {}

            def emit_xt_load(b, ns, split=False):
                xt = xin.tile([128, KC, QW], sdt, tag="xt", name=f"xt{b}{ns}")
                cols = slice(b * N + ns, b * N + ns + QW)
                if split:
                    nc.sync.dma_start(
                        xt[:, 0 : KC // 2, :],
                        xTr[0 : KC // 2, :, cols].rearrange("c p n -> p c n"),
                    )
                    nc.sync.dma_start(
                        xt[:, KC // 2 : KC, :],
                        xTr[KC // 2 : KC, :, cols].rearrange("c p n -> p c n"),
                    )
                else:
                    nc.sync.dma_start(
                        xt[:], xTr[:, :, cols].rearrange("c p n -> p c n")
                    )
                xts[(b, ns)] = xt

            # ---- emitters ----
            def emit_proj_chunk(b, ns, which, half):
                """8 accumulating matmuls + DVE bias-add for one 512-wide
                half of one weight set (q|kv) of one n-tile.  psum: psot."""
                wsb, dst, bias = (
                    (wq_sb, QT[b], bq_sb) if which == 0 else (wkv_sb, KVT[b], bkv_sb)
                )
                xt = xts[(b, ns)]
                sl = slice(half * PSD, (half + 1) * PSD)
                ps = psot.tile([128, PSD], F32, tag="av", name="projps")
                for c in range(KC):
                    nc.tensor.matmul(
                        ps[:], wsb[:, c, :], xt[:, c, sl],
                        start=(c == 0), stop=(c == KC - 1),
                    )
                nc.vector.tensor_scalar_add(
                    dst[:, ns + half * PSD : ns + (half + 1) * PSD], ps[:], bias[:]
                )

            def emit_kt2(b, half):
                kv_blk = KVT[b][64:128, half * QW : (half + 1) * QW].rearrange(
                    "p (k c) -> p k c", c=128
                )
                k0 = half * (KTS // 2)
                nc.sync.dma_start(KT2[b][0:64, k0 : k0 + KTS // 2, :], kv_blk)
                nc.sync.dma_start(KT2[b][64:128, k0 : k0 + KTS // 2, :], kv_blk)

            def emit_transpose_pair(b, kt0):
                for kt in (kt0, kt0 + 1):
                    vps = psot.tile([128, 64], sdt, tag="av", name="vps")
                    nc.tensor.transpose(
                        vps[:], KVT[b][0:64, kt * 128 : (kt + 1) * 128], ident[:]
                    )
                    nc.vector.tensor_copy(VO[b][:, kt, 0:64], vps[:])

            def emit_oproj_chunk(b, qs, nt, mh):
                ns = qs + nt * 128
                ops = psot.tile([128, PSD], F32, tag="av", name="oprojps")
                nc.tensor.matmul(
                    ops[:], OT[b][:, ns // 128, :],
                    wo_sb[:, mh * PSD : (mh + 1) * PSD],
                )
                osb = outp.tile([128, PSD], F32, tag="osb", name="oosb")
                # DVE evacuation (GPSIMD cannot read PSUM)
                nc.vector.tensor_copy(osb[:], ops[:])
                nc.sync.dma_start(
                    out[b * N + ns : b * N + ns + 128,
                        mh * PSD : (mh + 1) * PSD],
                    osb[:],
                )

            # ---- attention emitters ----
            pts = {}      # (ti, kt, h) -> P tile awaiting AV
            o_ps = {}     # (ti, h) -> psum accumulator

            def emit_st_exp(ti, kt, h):
                b, qs = tiles[ti]
                st = stp.tile([128, QW], F32, tag=f"st{h}", name=f"st{h}")
                for h2 in range(2):
                    sl = slice(h2 * PSD, (h2 + 1) * PSD)
                    nc.tensor.matmul(
                        st[:, sl],
                        KT2[b][64 * h : 64 * h + 64, kt, :],
                        QT[b][64 * h : 64 * h + 64,
                              qs + h2 * PSD : qs + (h2 + 1) * PSD],
                    )
                pt = ptp.tile([128, QW], sdt, tag="pt", name="pt")
                nc.scalar.activation(pt[:], st[:], AF.Exp, scale=SCALE)
                pts[(ti, kt, h)] = pt

            def emit_av(ti, kt, h):
                b, qs = tiles[ti]
                if kt == 0 and h == 0:
                    # both accumulators allocated back-to-back: consecutive
                    # ring slots -> always distinct psum banks
                    o_ps[(ti, 0)] = psot.tile([65, QW], F32, tag="av",
                                              name="avac0")
                    o_ps[(ti, 1)] = psot.tile([65, QW], F32, tag="av",
                                              name="avac1")
                acc = o_ps[(ti, h)]
                pt = pts.pop((ti, kt, h))
                for h2 in range(2):
                    sl = slice(h2 * PSD, (h2 + 1) * PSD)
                    nc.tensor.matmul(
                        acc[:, sl], VO[b][:, kt, :], pt[:, sl],
                        start=(kt == 0), stop=(kt == KTS - 1),
                    )

            def emit_evac_norm(ti):
                """Evacuate AV accumulators (frees psot ring) + normalize.
                DVE + Pool only; nothing on ACT."""
                b, qs = tiles[ti]
                q0 = qs // 128
                osbs = []
                for h in range(2):
                    osb = stat.tile([65, QW], F32, tag=f"osb{h}", name=f"osb{h}")
                    # DVE evacuation (GPSIMD cannot read PSUM)
                    nc.vector.tensor_copy(osb[:], o_ps.pop((ti, h))[:])
                    osbs.append(osb)
                for h in range(2):
                    osb = osbs[h]
                    # custom DVE ops need base partition 0: stage sums row
                    ssb = stat.tile([1, QW], F32, tag="ssb", name="ssb", bufs=1)
                    nc.vector.tensor_copy(ssb[:], osb[64:65, :])
                    r = stat.tile([1, QW], F32, tag="r", name="r", bufs=1)
                    nc.vector.reciprocal_approx_fast(r[:], ssb[:])
                    rb = stat.tile([64, QW], F32, tag="rb", name="rb", bufs=1)
                    nc.gpsimd.partition_broadcast(rb[:], r[0:1, :])
                    if h == 0:
                        nc.vector.tensor_mul(
                            OT[b][0:64, q0 : q0 + QW // 128, :],
                            osb[0:64, :].rearrange("p (k c) -> p k c", c=128),
                            rb[:].rearrange("p (k c) -> p k c", c=128),
                        )
                    else:
                        tmp = stat.tile([64, QW], sdt, tag="tmp", name="tmp",
                                        bufs=1)
                        nc.gpsimd.tensor_mul(tmp[:], osb[0:64, :], rb[:])
                        nc.sync.dma_start(
                            OT[b][64:128, q0 : q0 + QW // 128, :],
                            tmp[:].rearrange("p (k c) -> p k c", c=128),
                        )

            # ---- fill regions: work interleaved into each tile's kt loop.
            # Region ti must fit that tile's PE slack (~10us = ~40 matmuls);
            # leftovers roll forward.  Each unit: (n_mms, emit_fn).
            regions = {ti: deque() for ti in range(NT + 1)}

            def region_add(ti, n_mms, fn):
                regions[ti].append((n_mms, fn))

            # region 0 (during b0/qt0): KVT(b0) 2nd half + QT(b0,qt1) + b1 x
            region_add(0, 0, lambda: emit_xt_load(1, 0))
            for half in range(2):
                region_add(0, 8, lambda h=half: emit_proj_chunk(0, QW, 1, h))
            region_add(0, 0, lambda: emit_kt2(0, 1))
            for kt0 in range(KTS // 2, KTS, 2):
                region_add(0, 2, lambda k=kt0: emit_transpose_pair(0, k))
            for half in range(2):
                region_add(0, 8, lambda h=half: emit_proj_chunk(0, QW, 0, h))
            region_add(0, 0, lambda: emit_xt_load(1, QW))

            # region 1 (during b0/qt1): b1 first half + QT(b1,qt0)
            for half in range(2):
                region_add(1, 8, lambda h=half: emit_proj_chunk(1, 0, 1, h))
            region_add(1, 0, lambda: emit_kt2(1, 0))
            for kt0 in range(0, KTS // 2, 2):
                region_add(1, 2, lambda k=kt0: emit_transpose_pair(1, k))
            for half in range(2):
                region_add(1, 8, lambda h=half: emit_proj_chunk(1, 0, 0, h))

            # region 2 (during b1/qt0): b1 second half + QT(b1,qt1)
            for half in range(2):
                region_add(2, 8, lambda h=half: emit_proj_chunk(1, QW, 1, h))
            region_add(2, 0, lambda: emit_kt2(1, 1))
            for kt0 in range(KTS // 2, KTS, 2):
                region_add(2, 2, lambda k=kt0: emit_transpose_pair(1, k))
            for half in range(2):
                region_add(2, 8, lambda h=half: emit_proj_chunk(1, QW, 0, h))

            # region 3 (during b1/qt1): o_proj of tiles 0..2
            for ti in (0, 1, 2):
                b, qs = tiles[ti]
                for nt in range(QW // 128):
                    for mh in range(2):
                        region_add(3, 1, lambda b=b, q=qs, n=nt, m=mh:
                                   emit_oproj_chunk(b, q, n, m))

            # region 4 (tail): o_proj of tile 3
            b, qs = tiles[3]
            for nt in range(QW // 128):
                for mh in range(2):
                    region_add(4, 1, lambda b=b, q=qs, n=nt, m=mh:
                               emit_oproj_chunk(b, q, n, m))

            # ---- prologue: minimal work before the kt loop ----
            emit_xt_load(0, 0, split=True)
            emit_xt_load(0, QW)
            for half in range(2):
                emit_proj_chunk(0, 0, 1, half)      # KVT(b0, ns0)
            emit_kt2(0, 0)
            for kt0 in range(0, KTS // 2, 2):
                emit_transpose_pair(0, kt0)
            for half in range(2):
                emit_proj_chunk(0, 0, 0, half)      # QT(b0, qt0)

            # ---- main ACT-paced loop ----
            av_q = deque()            # pending (ti, kt, h) AV head-units

            def drain_one():
                emit_av(*av_q.popleft())

            for ti in range(NT):
                # psot discipline: fills may allocate psot only BEFORE this
                # tile's AV accumulators are allocated (fill window); once
                # the first AV is emitted, psot is owned by the accumulators
                # until the boundary evac.
                fill_window = True
                for kt in range(KTS):
                    if kt == 0:
                        # boundary: keep ACT fed first, then drain + evac
                        emit_st_exp(ti, 0, 0)
                        emit_st_exp(ti, 0, 1)
                        if ti > 0:
                            while av_q:
                                drain_one()
                            emit_evac_norm(ti - 1)
                        av_q.append((ti, 0, 0))
                        av_q.append((ti, 0, 1))
                        continue
                    emit_st_exp(ti, kt, 0)
                    emit_st_exp(ti, kt, 1)
                    if fill_window and regions[ti] \
                            and len(av_q) // 2 < CAP_KTS:
                        # AV paused: the whole ACT window minus ST is fill
                        budget = 1594.0
                        while budget > 0 and regions[ti] \
                                and len(av_q) // 2 < CAP_KTS:
                            n, fn = regions[ti].popleft()
                            fn()
                            budget -= max(n, 1) * 241.0
                    else:
                        fill_window = False
                        budget = SLOT_FILL_NS
                        emitted = False
                        while budget > 0 and len(av_q) > 2:
                            drain_one()
                            budget -= 482.0
                            emitted = True
                        if not emitted and FILLER:
                            for _ in range(FILLER):
                                nc.tensor.ldweights(ident[:, 0:1])
                    av_q.append((ti, kt, 0))
                    av_q.append((ti, kt, 1))
                regions[ti + 1].extendleft(reversed(regions[ti]))
                regions[ti].clear()

            # final boundary + tail
            while av_q:
                drain_one()
            emit_evac_norm(NT - 1)
            while regions[NT]:
                n, fn = regions[NT].popleft()
                fn()

            assert not pts and not o_ps

    nc.compile()
    return nc


def _get_nc(mode):
    key = (mode, P_DEPTH, SLOT_FILL_NS, FILLER)
    if key not in _NC_CACHE:
        _NC_CACHE[key] = _build_program(mode)
    return _NC_CACHE[key]


def _prep_in_maps(inputs, mode):
    ndt = _np_dt(mode)
    x = np.asarray(inputs["x"], np.float32)
    Wq = np.asarray(inputs["Wq"], np.float32)
    bq = np.asarray(inputs["bq"], np.float32)
    Wk = np.asarray(inputs["Wk"], np.float32)
    bk = np.asarray(inputs["bk"], np.float32)
    Wv = np.asarray(inputs["Wv"], np.float32)
    bv = np.asarray(inputs["bv"], np.float32)
    Wo = np.asarray(inputs["Wo"], np.float32)

    xT = np.ascontiguousarray(x.reshape(BN, D).T).astype(ndt)
    in_maps = []
    for i in range(NCORES):
        j0 = i * JC              # query-head column offset (heads 2i, 2i+1)
        g = i // 2               # kv head for this core
        v0 = g * HD
        wkv_i = np.concatenate(
            [Wv[:, v0 : v0 + HD], Wk[:, v0 : v0 + HD]], axis=1
        )  # V cols first (rows 0:64 of KVT), K cols second (rows 64:128)
        bkv_i = np.concatenate([bv[v0 : v0 + HD], bk[v0 : v0 + HD]])
        in_maps.append({
            "xT": xT,
            "wq": np.ascontiguousarray(Wq[:, j0 : j0 + JC]).astype(ndt),
            "wkv": np.ascontiguousarray(wkv_i).astype(ndt),
            "wo": np.ascontiguousarray(Wo[j0 : j0 + JC, :]).astype(ndt),
            "bq": np.ascontiguousarray(bq[j0 : j0 + JC]).reshape(JC, 1)
                    .astype(np.float32),
            "bkv": np.ascontiguousarray(bkv_i).reshape(JC, 1).astype(np.float32),
            "ident": np.eye(64, dtype=np.float32).astype(ndt),
            "ones": np.ones((128, KTS), dtype=np.float32).astype(ndt),
        })
    return in_maps


def _run(inputs, trace=False):
    mode = MM_MODE
    nc = _get_nc(mode)
    in_maps = _prep_in_maps(inputs, mode)
    res = run_bass_kernel_spmd(
        nc, in_maps, core_ids=list(range(NCORES)), trace=trace
    )
    bo = np.asarray(inputs["bo"], np.float32)
    acc = res.results[0]["out"].astype(np.float64)
    for i in range(1, NCORES):
        acc += res.results[i]["out"].astype(np.float64)
    full = (acc + bo.astype(np.float64)).astype(np.float32).reshape(B, N, D)
    return full, res


def kernel(**inputs):
    return _run(inputs, trace=False)[0]


# revision 14
# speedup vs baseline: 1.0603x; 1.0603x over previous
"""8-core tensor-parallel GQA attention for TRN2 — ACT-paced schedule.

Problem: x[2,2048,1024] -> QKV proj -> 16-head attention (4 KV heads,
GQA groups of 4) -> out proj.  Sharding: 2 query heads + their shared
KV head per core; o_proj row-parallel with host-side partial-sum
reduce (host reduce is free for HW exec time).

Engine budget per core (the floor):
  ACT: 128 exp tiles [128,1024] ~1.04us each  -> ~133us  (hard floor;
       exp exists only on the Activation engine)
  PE:  ~295k cycles @2.4GHz                   -> ~124us
Everything else (DVE/Pool/DMA) is kept off those two engines.

Schedule: the kt loop is paced by ACT (2 exps/kt, 2076ns).  PE per kt
does the ST pair (row-tiled, both heads in disjoint PE row groups,
~480ns) + AV pair (~960ns), leaving ~630ns/kt of slack that absorbs
the projection / o_proj "fill" work.  PSUM (8 banks):
  stA, stB: one [128,1024] f32 tile per head (4 banks).  bufs=1 each
       makes the WAR chain ST_kt+1 <- exp_kt explicit.
  psot: ring of 2x4KB slots (4 banks) serving the AV accumulators
       [65,1024] AND the proj/o_proj fill chunks.  Both accumulators
       are allocated back-to-back so they always land in distinct
       slots.
A deep SBUF pool of P tiles (exp outputs) lets AV run several kt
behind exp, so fill bursts don't stall ACT: ST/exp continue at full
cadence while AV + fills share the PE.  The AV backlog cap tightens
near each q-tile boundary so the boundary drain stays short.

The [V|1] stationary trick accumulates the softmax denominator in
row 64 of the AV accumulator for free; normalize = recip + partition
broadcast + mul on DVE/Pool only.
"""

import os
import sys
from collections import deque

import numpy as np

for _p in ("/opt/trn_rl_repo", "/root/.axon_site/_ro/trn_rl_repo"):
    if os.path.isdir(_p) and _p not in sys.path:
        sys.path.append(_p)

import concourse.bass as bass
import concourse.tile as tile
from concourse import bacc, mybir
from concourse.bass_utils import run_bass_kernel_spmd

AF = mybir.ActivationFunctionType
F32 = mybir.dt.float32

B, N, D = 2, 2048, 1024
BN = B * N
HEADS, KV_HEADS, HD = 16, 4, 64
SCALE = HD ** -0.5
NCORES = 8
HPC = HEADS // NCORES          # query heads per core = 2
JC = HPC * HD                  # per-core head-dim columns = 128
KC = D // 128                  # contraction chunks for projections = 8
PSD = 512                      # psum bank size in f32 / matmul col cap
QW = 1024                      # attention q-tile width
KTS = N // 128                 # key tiles per batch = 16
NQT = N // QW                  # q tiles per batch = 2

MM_MODE = os.environ.get("KERNEL_MM_DTYPE", "bfloat16")
P_DEPTH = int(os.environ.get("KERNEL_P_DEPTH", "22"))     # P-tile ring
CAP_KTS = P_DEPTH // 2 - 1                                # max AV lag in kts
SLOT_FILL_NS = float(os.environ.get("KERNEL_SLOT_FILL", "1100"))
FILLER = int(os.environ.get("KERNEL_FILLER", "2"))        # dummy ldw pad

_NC_CACHE: dict[tuple, object] = {}


def _storage_dt(mode):
    if mode == "bfloat16":
        return mybir.dt.bfloat16
    if mode == "float32r":
        return mybir.dt.float32r
    return F32


def _np_dt(mode):
    if mode == "bfloat16":
        import ml_dtypes
        return ml_dtypes.bfloat16
    return np.float32


def _build_program(mode):
    sdt = _storage_dt(mode)
    nc = bacc.Bacc("TRN2", target_bir_lowering=False, debug=False)

    xT = nc.dram_tensor("xT", [D, BN], sdt, kind="ExternalInput")
    wq = nc.dram_tensor("wq", [D, JC], sdt, kind="ExternalInput")
    wkv = nc.dram_tensor("wkv", [D, JC], sdt, kind="ExternalInput")
    wo = nc.dram_tensor("wo", [JC, D], sdt, kind="ExternalInput")
    bq = nc.dram_tensor("bq", [JC, 1], F32, kind="ExternalInput")
    bkv = nc.dram_tensor("bkv", [JC, 1], F32, kind="ExternalInput")
    ident_d = nc.dram_tensor("ident", [64, 64], sdt, kind="ExternalInput")
    ones_d = nc.dram_tensor("ones", [128, KTS], sdt, kind="ExternalInput")
    # bf16 partials: halves write traffic + enables DVE 2x evacuation;
    # host sums partials in f64 so the extra error is ~0.2% rms
    out = nc.dram_tensor("out", [BN, D], sdt, kind="ExternalOutput")

    xTr = xT[:].rearrange("(c p) n -> c p n", p=128)
    wqr = wq[:].rearrange("(c p) j -> c p j", p=128)
    wkvr = wkv[:].rearrange("(c p) j -> c p j", p=128)

    # q-tiles in execution order
    tiles = [(b, qt * QW) for b in range(B) for qt in range(NQT)]
    NT = len(tiles)

    with tile.TileContext(nc) as tc:
        with (
            tc.tile_pool(name="consts", bufs=1) as consts,
            tc.tile_pool(name="xin", bufs=3) as xin,
            tc.tile_pool(name="big", bufs=1) as big,
            tc.tile_pool(name="ptp", bufs=P_DEPTH) as ptp,
            tc.tile_pool(name="stat", bufs=2) as stat,
            tc.tile_pool(name="outp", bufs=6) as outp,
            tc.tile_pool(name="stp", bufs=1, space="PSUM") as stp,
            tc.tile_pool(name="psot", bufs=2, space="PSUM") as psot,
        ):
            wq_sb = consts.tile([128, KC, 128], sdt, tag="wq")
            wkv_sb = consts.tile([128, KC, 128], sdt, tag="wkv")
            wo_sb = consts.tile([128, D], sdt, tag="wo")
            bq_sb = consts.tile([128, 1], F32, tag="bq")
            bkv_sb = consts.tile([128, 1], F32, tag="bkv")
            ident = consts.tile([64, 64], sdt, tag="ident")

            QT, KVT, KT2, VO, OT = {}, {}, {}, {}, {}
            for b in range(B):
                QT[b] = big.tile([128, N], sdt, tag=f"QT{b}", name=f"QT{b}")
                KVT[b] = big.tile([128, N], sdt, tag=f"KVT{b}", name=f"KVT{b}")
                KT2[b] = big.tile([128, KTS, 128], sdt, tag=f"KT2{b}",
                                  name=f"KT2{b}")
                VO[b] = big.tile([128, KTS, 65], sdt, tag=f"VO{b}", name=f"VO{b}")
                OT[b] = big.tile([128, N // 128, 128], sdt, tag=f"OT{b}",
                                 name=f"OT{b}")

            # constants: batched 3D-AP DMAs in criticality order (SWDGE
            # issue is ~1us per dma_start).  KVT proj is first compute, so
            # wkv leads; small tiles ride the idle DVE HWDGE queue.
            nc.gpsimd.dma_start(
                wkv_sb[:], wkv[:].rearrange("(c p) j -> p c j", p=128)
            )
            nc.gpsimd.dma_start(
                wq_sb[:], wq[:].rearrange("(c p) j -> p c j", p=128)
            )
            for b in range(B):
                nc.gpsimd.dma_start(
                    VO[b][:, :, 64:65],
                    ones_d[:].rearrange("p (k o) -> p k o", o=1),
                )
            nc.gpsimd.dma_start(wo_sb[:], wo[:])
            # ACT queue is idle in the prologue; 3 small issues end long
            # before the first exp
            nc.scalar.dma_start(bkv_sb[:], bkv[:])
            nc.scalar.dma_start(bq_sb[:], bq[:])
            nc.scalar.dma_start(ident[:], ident_d[:])

            # ---- x loads: one DMA per [128, KC, 1024] tile (3D AP); the
            # first tile is split in half so compute starts after ~1MB
            xts = {}

            def emit_xt_load(b, ns, split=False):
                xt = xin.tile([128, KC, QW], sdt, tag="xt", name=f"xt{b}{ns}")
                cols = slice(b * N + ns, b * N + ns + QW)
                if split:
                    nc.sync.dma_start(
                        xt[:, 0 : KC // 2, :],
                        xTr[0 : KC // 2, :, cols].rearrange("c p n -> p c n"),
                    )
                    nc.sync.dma_start(
                        xt[:, KC // 2 : KC, :],
                        xTr[KC // 2 : KC, :, cols].rearrange("c p n -> p c n"),
                    )
                else:
                    nc.sync.dma_start(
                        xt[:], xTr[:, :, cols].rearrange("c p n -> p c n")
                    )
                xts[(b, ns)] = xt

            # ---- emitters ----
            def emit_proj_chunk(b, ns, which, half):
                """8 accumulating matmuls + DVE bias-add for one 512-wide
                half of one weight set (q|kv) of one n-tile.  psum: psot."""
                wsb, dst, bias = (
                    (wq_sb, QT[b], bq_sb) if which == 0 else (wkv_sb, KVT[b], bkv_sb)
                )
                xt = xts[(b, ns)]
                sl = slice(half * PSD, (half + 1) * PSD)
                ps = psot.tile([128, PSD], F32, tag="av", name="projps")
                for c in range(KC):
                    nc.tensor.matmul(
                        ps[:], wsb[:, c, :], xt[:, c, sl],
                        start=(c == 0), stop=(c == KC - 1),
                    )
                nc.vector.tensor_scalar_add(
                    dst[:, ns + half * PSD : ns + (half + 1) * PSD], ps[:], bias[:]
                )

            def emit_kt2(b, half):
                kv_blk = KVT[b][64:128, half * QW : (half + 1) * QW].rearrange(
                    "p (k c) -> p k c", c=128
                )
                k0 = half * (KTS // 2)
                nc.sync.dma_start(KT2[b][0:64, k0 : k0 + KTS // 2, :], kv_blk)
                nc.sync.dma_start(KT2[b][64:128, k0 : k0 + KTS // 2, :], kv_blk)

            def emit_transpose_pair(b, kt0):
                for kt in (kt0, kt0 + 1):
                    vps = psot.tile([128, 64], sdt, tag="av", name="vps")
                    nc.tensor.transpose(
                        vps[:], KVT[b][0:64, kt * 128 : (kt + 1) * 128], ident[:]
                    )
                    nc.vector.tensor_copy(VO[b][:, kt, 0:64], vps[:])

            def emit_oproj_chunk(b, qs, nt, mh):
                ns = qs + nt * 128
                ops = psot.tile([128, PSD], F32, tag="av", name="oprojps")
                nc.tensor.matmul(
                    ops[:], OT[b][:, ns // 128, :],
                    wo_sb[:, mh * PSD : (mh + 1) * PSD],
                )
                osb = outp.tile([128, PSD], sdt, tag="osb", name="oosb")
                # DVE evacuation (GPSIMD cannot read PSUM); bf16 out
                nc.vector.tensor_copy(osb[:], ops[:])
                nc.sync.dma_start(
                    out[b * N + ns : b * N + ns + 128,
                        mh * PSD : (mh + 1) * PSD],
                    osb[:],
                )

            # ---- attention emitters ----
            pts = {}      # (ti, kt, h) -> P tile awaiting AV
            o_ps = {}     # (ti, h) -> psum accumulator

            def emit_st_exp(ti, kt, h):
                b, qs = tiles[ti]
                st = stp.tile([128, QW], F32, tag=f"st{h}", name=f"st{h}")
                for h2 in range(2):
                    sl = slice(h2 * PSD, (h2 + 1) * PSD)
                    nc.tensor.matmul(
                        st[:, sl],
                        KT2[b][64 * h : 64 * h + 64, kt, :],
                        QT[b][64 * h : 64 * h + 64,
                              qs + h2 * PSD : qs + (h2 + 1) * PSD],
                    )
                pt = ptp.tile([128, QW], sdt, tag="pt", name="pt")
                nc.scalar.activation(pt[:], st[:], AF.Exp, scale=SCALE)
                pts[(ti, kt, h)] = pt

            def emit_av(ti, kt, h):
                b, qs = tiles[ti]
                if kt == 0 and h == 0:
                    # both accumulators allocated back-to-back: consecutive
                    # ring slots -> always distinct psum banks
                    o_ps[(ti, 0)] = psot.tile([65, QW], F32, tag="av",
                                              name="avac0")
                    o_ps[(ti, 1)] = psot.tile([65, QW], F32, tag="av",
                                              name="avac1")
                acc = o_ps[(ti, h)]
                pt = pts.pop((ti, kt, h))
                for h2 in range(2):
                    sl = slice(h2 * PSD, (h2 + 1) * PSD)
                    nc.tensor.matmul(
                        acc[:, sl], VO[b][:, kt, :], pt[:, sl],
                        start=(kt == 0), stop=(kt == KTS - 1),
                    )

            def emit_evac_norm(ti):
                """Evacuate AV accumulators (frees psot ring) + normalize.
                DVE + Pool only; nothing on ACT."""
                b, qs = tiles[ti]
                q0 = qs // 128
                osbs = []
                for h in range(2):
                    osb = stat.tile([65, QW], F32, tag=f"osb{h}", name=f"osb{h}")
                    # DVE evacuation (GPSIMD cannot read PSUM)
                    nc.vector.tensor_copy(osb[:], o_ps.pop((ti, h))[:])
                    osbs.append(osb)
                for h in range(2):
                    osb = osbs[h]
                    # custom DVE ops need base partition 0: stage sums row
                    ssb = stat.tile([1, QW], F32, tag="ssb", name="ssb", bufs=1)
                    nc.vector.tensor_copy(ssb[:], osb[64:65, :])
                    r = stat.tile([1, QW], F32, tag="r", name="r", bufs=1)
                    nc.vector.reciprocal_approx_fast(r[:], ssb[:])
                    rb = stat.tile([64, QW], F32, tag="rb", name="rb", bufs=1)
                    nc.gpsimd.partition_broadcast(rb[:], r[0:1, :])
                    if h == 0:
                        nc.vector.tensor_mul(
                            OT[b][0:64, q0 : q0 + QW // 128, :],
                            osb[0:64, :].rearrange("p (k c) -> p k c", c=128),
                            rb[:].rearrange("p (k c) -> p k c", c=128),
                        )
                    else:
                        tmp = stat.tile([64, QW], sdt, tag="tmp", name="tmp",
                                        bufs=1)
                        nc.gpsimd.tensor_mul(tmp[:], osb[0:64, :], rb[:])
                        nc.sync.dma_start(
                            OT[b][64:128, q0 : q0 + QW // 128, :],
                            tmp[:].rearrange("p (k c) -> p k c", c=128),
                        )

            # ---- fill regions: work interleaved into each tile's kt loop.
            # Region ti must fit that tile's PE slack (~10us = ~40 matmuls);
            # leftovers roll forward.  Each unit: (n_mms, emit_fn).
            regions = {ti: deque() for ti in range(NT + 1)}

            def region_add(ti, n_mms, fn):
                regions[ti].append((n_mms, fn))

            # region 0 (during b0/qt0): KVT(b0) 2nd half + QT(b0,qt1) + b1 x
            region_add(0, 0, lambda: emit_xt_load(1, 0))
            for half in range(2):
                region_add(0, 8, lambda h=half: emit_proj_chunk(0, QW, 1, h))
            region_add(0, 0, lambda: emit_kt2(0, 1))
            for kt0 in range(KTS // 2, KTS, 2):
                region_add(0, 2, lambda k=kt0: emit_transpose_pair(0, k))
            for half in range(2):
                region_add(0, 8, lambda h=half: emit_proj_chunk(0, QW, 0, h))
            region_add(0, 0, lambda: emit_xt_load(1, QW))

            # region 1 (during b0/qt1): b1 first half + QT(b1,qt0)
            for half in range(2):
                region_add(1, 8, lambda h=half: emit_proj_chunk(1, 0, 1, h))
            region_add(1, 0, lambda: emit_kt2(1, 0))
            for kt0 in range(0, KTS // 2, 2):
                region_add(1, 2, lambda k=kt0: emit_transpose_pair(1, k))
            for half in range(2):
                region_add(1, 8, lambda h=half: emit_proj_chunk(1, 0, 0, h))

            # region 2 (during b1/qt0): b1 second half + QT(b1,qt1)
            for half in range(2):
                region_add(2, 8, lambda h=half: emit_proj_chunk(1, QW, 1, h))
            region_add(2, 0, lambda: emit_kt2(1, 1))
            for kt0 in range(KTS // 2, KTS, 2):
                region_add(2, 2, lambda k=kt0: emit_transpose_pair(1, k))
            for half in range(2):
                region_add(2, 8, lambda h=half: emit_proj_chunk(1, QW, 0, h))

            # o_proj of tile ti rides the fill window of tile ti+2 (its OT
            # is ready early in tile ti+1); tile 2's o_proj shares region 3,
            # tile 3's runs in the tail
            for ti, rgn in ((0, 2), (1, 3), (2, 3), (3, 4)):
                b, qs = tiles[ti]
                for nt in range(QW // 128):
                    for mh in range(2):
                        region_add(rgn, 1, lambda b=b, q=qs, n=nt, m=mh:
                                   emit_oproj_chunk(b, q, n, m))

            # ---- prologue: minimal work before the kt loop ----
            emit_xt_load(0, 0, split=True)
            emit_xt_load(0, QW)
            for half in range(2):
                emit_proj_chunk(0, 0, 1, half)      # KVT(b0, ns0)
            emit_kt2(0, 0)
            for kt0 in range(0, KTS // 2, 2):
                emit_transpose_pair(0, kt0)
            for half in range(2):
                emit_proj_chunk(0, 0, 0, half)      # QT(b0, qt0)

            # ---- main ACT-paced loop ----
            av_q = deque()            # pending (ti, kt, h) AV head-units

            def drain_one():
                emit_av(*av_q.popleft())

            for ti in range(NT):
                # psot discipline: fills may allocate psot only BEFORE this
                # tile's AV accumulators are allocated (fill window); once
                # the first AV is emitted, psot is owned by the accumulators
                # until the boundary evac.
                fill_window = True
                for kt in range(KTS):
                    if kt == 0:
                        # boundary: keep ACT fed first, then drain + evac
                        emit_st_exp(ti, 0, 0)
                        emit_st_exp(ti, 0, 1)
                        if ti > 0:
                            while av_q:
                                drain_one()
                            emit_evac_norm(ti - 1)
                        av_q.append((ti, 0, 0))
                        av_q.append((ti, 0, 1))
                        continue
                    emit_st_exp(ti, kt, 0)
                    emit_st_exp(ti, kt, 1)
                    if fill_window and regions[ti] \
                            and len(av_q) // 2 < CAP_KTS:
                        # AV paused: the whole ACT window minus ST is fill
                        budget = 1594.0
                        while budget > 0 and regions[ti] \
                                and len(av_q) // 2 < CAP_KTS:
                            n, fn = regions[ti].popleft()
                            fn()
                            budget -= max(n, 1) * 241.0
                    else:
                        fill_window = False
                        budget = SLOT_FILL_NS
                        emitted = False
                        while budget > 0 and len(av_q) > 2:
                            drain_one()
                            budget -= 482.0
                            emitted = True
                        if not emitted and FILLER:
                            for _ in range(FILLER):
                                nc.tensor.ldweights(ident[:, 0:1])
                    av_q.append((ti, kt, 0))
                    av_q.append((ti, kt, 1))
                regions[ti + 1].extendleft(reversed(regions[ti]))
                regions[ti].clear()

            # final boundary + tail
            while av_q:
                drain_one()
            emit_evac_norm(NT - 1)
            while regions[NT]:
                n, fn = regions[NT].popleft()
                fn()

            assert not pts and not o_ps

    nc.compile()
    return nc


def _get_nc(mode):
    key = (mode, P_DEPTH, SLOT_FILL_NS, FILLER)
    if key not in _NC_CACHE:
        _NC_CACHE[key] = _build_program(mode)
    return _NC_CACHE[key]


def _prep_in_maps(inputs, mode):
    ndt = _np_dt(mode)
    x = np.asarray(inputs["x"], np.float32)
    Wq = np.asarray(inputs["Wq"], np.float32)
    bq = np.asarray(inputs["bq"], np.float32)
    Wk = np.asarray(inputs["Wk"], np.float32)
    bk = np.asarray(inputs["bk"], np.float32)
    Wv = np.asarray(inputs["Wv"], np.float32)
    bv = np.asarray(inputs["bv"], np.float32)
    Wo = np.asarray(inputs["Wo"], np.float32)

    xT = np.ascontiguousarray(x.reshape(BN, D).T).astype(ndt)
    in_maps = []
    for i in range(NCORES):
        j0 = i * JC              # query-head column offset (heads 2i, 2i+1)
        g = i // 2               # kv head for this core
        v0 = g * HD
        wkv_i = np.concatenate(
            [Wv[:, v0 : v0 + HD], Wk[:, v0 : v0 + HD]], axis=1
        )  # V cols first (rows 0:64 of KVT), K cols second (rows 64:128)
        bkv_i = np.concatenate([bv[v0 : v0 + HD], bk[v0 : v0 + HD]])
        in_maps.append({
            "xT": xT,
            "wq": np.ascontiguousarray(Wq[:, j0 : j0 + JC]).astype(ndt),
            "wkv": np.ascontiguousarray(wkv_i).astype(ndt),
            "wo": np.ascontiguousarray(Wo[j0 : j0 + JC, :]).astype(ndt),
            "bq": np.ascontiguousarray(bq[j0 : j0 + JC]).reshape(JC, 1)
                    .astype(np.float32),
            "bkv": np.ascontiguousarray(bkv_i).reshape(JC, 1).astype(np.float32),
            "ident": np.eye(64, dtype=np.float32).astype(ndt),
            "ones": np.ones((128, KTS), dtype=np.float32).astype(ndt),
        })
    return in_maps


def _run(inputs, trace=False):
    mode = MM_MODE
    nc = _get_nc(mode)
    in_maps = _prep_in_maps(inputs, mode)
    res = run_bass_kernel_spmd(
        nc, in_maps, core_ids=list(range(NCORES)), trace=trace
    )
    bo = np.asarray(inputs["bo"], np.float32)
    acc = res.results[0]["out"].astype(np.float64)
    for i in range(1, NCORES):
        acc += res.results[i]["out"].astype(np.float64)
    full = (acc + bo.astype(np.float64)).astype(np.float32).reshape(B, N, D)
    return full, res


def kernel(**inputs):
    return _run(inputs, trace=False)[0]


# revision 19
# speedup vs baseline: 1.1334x; 1.0690x over previous
"""8-core tensor-parallel GQA attention for TRN2 — ACT-paced schedule.

Problem: x[2,2048,1024] -> QKV proj -> 16-head attention (4 KV heads,
GQA groups of 4) -> out proj.  Sharding: 2 query heads + their shared
KV head per core; o_proj row-parallel with host-side partial-sum
reduce (host reduce is free for HW exec time).

Engine budget per core (the floor):
  ACT: 128 exp tiles [128,1024] ~1.04us each  -> ~133us  (hard floor;
       exp exists only on the Activation engine)
  PE:  ~295k cycles @2.4GHz                   -> ~124us
Everything else (DVE/Pool/DMA) is kept off those two engines.

Schedule: the kt loop is paced by ACT (2 exps/kt, 2076ns).  PE per kt
does the ST pair (row-tiled, both heads in disjoint PE row groups,
~480ns) + AV pair (~960ns), leaving ~630ns/kt of slack that absorbs
the projection / o_proj "fill" work.  PSUM (8 banks):
  stA, stB: one [128,1024] f32 tile per head (4 banks).  bufs=1 each
       makes the WAR chain ST_kt+1 <- exp_kt explicit.
  psot: ring of 2x4KB slots (4 banks) serving the AV accumulators
       [65,1024] AND the proj/o_proj fill chunks.  Both accumulators
       are allocated back-to-back so they always land in distinct
       slots.
A deep SBUF pool of P tiles (exp outputs) lets AV run several kt
behind exp, so fill bursts don't stall ACT: ST/exp continue at full
cadence while AV + fills share the PE.  The AV backlog cap tightens
near each q-tile boundary so the boundary drain stays short.

The [V|1] stationary trick accumulates the softmax denominator in
row 64 of the AV accumulator for free; normalize = recip + partition
broadcast + mul on DVE/Pool only.
"""

import os
import sys
from collections import deque

import numpy as np

for _p in ("/opt/trn_rl_repo", "/root/.axon_site/_ro/trn_rl_repo"):
    if os.path.isdir(_p) and _p not in sys.path:
        sys.path.append(_p)

import concourse.bass as bass
import concourse.tile as tile
from concourse import bacc, mybir
from concourse.bass_utils import run_bass_kernel_spmd

AF = mybir.ActivationFunctionType
F32 = mybir.dt.float32

B, N, D = 2, 2048, 1024
BN = B * N
HEADS, KV_HEADS, HD = 16, 4, 64
SCALE = HD ** -0.5
NCORES = 8
HPC = HEADS // NCORES          # query heads per core = 2
JC = HPC * HD                  # per-core head-dim columns = 128
KC = D // 128                  # contraction chunks for projections = 8
PSD = 512                      # psum bank size in f32 / matmul col cap
QW = 1024                      # attention q-tile width
KTS = N // 128                 # key tiles per batch = 16
NQT = N // QW                  # q tiles per batch = 2

MM_MODE = os.environ.get("KERNEL_MM_DTYPE", "bfloat16")
P_DEPTH = int(os.environ.get("KERNEL_P_DEPTH", "22"))     # P-tile ring
CAP_KTS = P_DEPTH // 2 - 1                                # max AV lag in kts
SLOT_FILL_NS = float(os.environ.get("KERNEL_SLOT_FILL", "1100"))
FILLER = int(os.environ.get("KERNEL_FILLER", "2"))        # dummy ldw pad

_NC_CACHE: dict[tuple, object] = {}


def _storage_dt(mode):
    if mode == "bfloat16":
        return mybir.dt.bfloat16
    if mode == "float32r":
        return mybir.dt.float32r
    return F32


def _np_dt(mode):
    if mode == "bfloat16":
        import ml_dtypes
        return ml_dtypes.bfloat16
    return np.float32


def _build_program(mode):
    sdt = _storage_dt(mode)
    nc = bacc.Bacc("TRN2", target_bir_lowering=False, debug=False)

    xT = nc.dram_tensor("xT", [D, BN], sdt, kind="ExternalInput")
    wq = nc.dram_tensor("wq", [D, JC], sdt, kind="ExternalInput")
    wkv = nc.dram_tensor("wkv", [D, JC], sdt, kind="ExternalInput")
    wo = nc.dram_tensor("wo", [JC, D], sdt, kind="ExternalInput")
    bq = nc.dram_tensor("bq", [JC, 1], F32, kind="ExternalInput")
    bkv = nc.dram_tensor("bkv", [JC, 1], F32, kind="ExternalInput")
    ident_d = nc.dram_tensor("ident", [64, 64], sdt, kind="ExternalInput")
    ones_d = nc.dram_tensor("ones", [128, KTS], sdt, kind="ExternalInput")
    # bf16 partials: halves write traffic + enables DVE 2x evacuation;
    # host sums partials in f64 so the extra error is ~0.2% rms
    out = nc.dram_tensor("out", [BN, D], sdt, kind="ExternalOutput")

    xTr = xT[:].rearrange("(c p) n -> c p n", p=128)
    wqr = wq[:].rearrange("(c p) j -> c p j", p=128)
    wkvr = wkv[:].rearrange("(c p) j -> c p j", p=128)

    # q-tiles in execution order
    tiles = [(b, qt * QW) for b in range(B) for qt in range(NQT)]
    NT = len(tiles)

    with tile.TileContext(nc) as tc:
        with (
            tc.tile_pool(name="consts", bufs=1) as consts,
            tc.tile_pool(name="xin", bufs=3) as xin,
            tc.tile_pool(name="big", bufs=1) as big,
            tc.tile_pool(name="ptp", bufs=P_DEPTH) as ptp,
            tc.tile_pool(name="stat", bufs=2) as stat,
            tc.tile_pool(name="outp", bufs=6) as outp,
            tc.tile_pool(name="stp", bufs=1, space="PSUM") as stp,
            tc.tile_pool(name="psot", bufs=2, space="PSUM") as psot,
        ):
            wq_sb = consts.tile([128, KC, 128], sdt, tag="wq")
            wkv_sb = consts.tile([128, KC, 128], sdt, tag="wkv")
            wo_sb = consts.tile([128, D], sdt, tag="wo")
            bq_sb = consts.tile([128, 1], F32, tag="bq")
            bkv_sb = consts.tile([128, 1], F32, tag="bkv")
            ident = consts.tile([64, 64], sdt, tag="ident")

            QT, KVT, KT2, VO, OT = {}, {}, {}, {}, {}
            for b in range(B):
                QT[b] = big.tile([128, N], sdt, tag=f"QT{b}", name=f"QT{b}")
                KVT[b] = big.tile([128, N], sdt, tag=f"KVT{b}", name=f"KVT{b}")
                KT2[b] = big.tile([128, KTS, 128], sdt, tag=f"KT2{b}",
                                  name=f"KT2{b}")
                VO[b] = big.tile([128, KTS, 65], sdt, tag=f"VO{b}", name=f"VO{b}")
                OT[b] = big.tile([128, N // 128, 128], sdt, tag=f"OT{b}",
                                 name=f"OT{b}")

            # constants: batched 3D-AP DMAs in criticality order.  The sync
            # queue carries small consts + weights (KVT proj needs wkv
            # first); x(b0) rides the otherwise-idle ACT HWDGE queue so the
            # two biggest prologue transfers use separate DMA queues.
            nc.sync.dma_start(bkv_sb[:], bkv[:])
            nc.sync.dma_start(bq_sb[:], bq[:])
            nc.sync.dma_start(ident[:], ident_d[:])
            nc.sync.dma_start(
                wkv_sb[:], wkv[:].rearrange("(c p) j -> p c j", p=128)
            )
            nc.sync.dma_start(
                wq_sb[:], wq[:].rearrange("(c p) j -> p c j", p=128)
            )
            for b in range(B):
                nc.gpsimd.dma_start(
                    VO[b][:, :, 64:65],
                    ones_d[:].rearrange("p (k o) -> p k o", o=1),
                )
            nc.gpsimd.dma_start(wo_sb[:], wo[:])

            # ---- x loads: one DMA per [128, KC, 1024] tile (3D AP); the
            # first tile is split in half so compute starts after ~1MB
            xts = {}

            def emit_xt_load(b, ns, split=False):
                xt = xin.tile([128, KC, QW], sdt, tag="xt", name=f"xt{b}{ns}")
                cols = slice(b * N + ns, b * N + ns + QW)
                # b0 (prologue-critical) on the ACT queue, b1 on sync
                eng = nc.scalar if b == 0 else nc.sync
                if split:
                    eng.dma_start(
                        xt[:, 0 : KC // 2, :],
                        xTr[0 : KC // 2, :, cols].rearrange("c p n -> p c n"),
                    )
                    eng.dma_start(
                        xt[:, KC // 2 : KC, :],
                        xTr[KC // 2 : KC, :, cols].rearrange("c p n -> p c n"),
                    )
                else:
                    eng.dma_start(
                        xt[:], xTr[:, :, cols].rearrange("c p n -> p c n")
                    )
                xts[(b, ns)] = xt

            # ---- emitters ----
            def emit_proj_chunk(b, ns, which, half):
                """8 accumulating matmuls + DVE bias-add for one 512-wide
                half of one weight set (q|kv) of one n-tile.  psum: psot."""
                wsb, dst, bias = (
                    (wq_sb, QT[b], bq_sb) if which == 0 else (wkv_sb, KVT[b], bkv_sb)
                )
                xt = xts[(b, ns)]
                sl = slice(half * PSD, (half + 1) * PSD)
                ps = psot.tile([128, PSD], F32, tag="av", name="projps")
                for c in range(KC):
                    nc.tensor.matmul(
                        ps[:], wsb[:, c, :], xt[:, c, sl],
                        start=(c == 0), stop=(c == KC - 1),
                    )
                nc.vector.tensor_scalar_add(
                    dst[:, ns + half * PSD : ns + (half + 1) * PSD], ps[:], bias[:]
                )

            def emit_kt2(b, half):
                kv_blk = KVT[b][64:128, half * QW : (half + 1) * QW].rearrange(
                    "p (k c) -> p k c", c=128
                )
                k0 = half * (KTS // 2)
                nc.sync.dma_start(KT2[b][0:64, k0 : k0 + KTS // 2, :], kv_blk)
                nc.sync.dma_start(KT2[b][64:128, k0 : k0 + KTS // 2, :], kv_blk)

            def emit_transpose_pair(b, kt0):
                for kt in (kt0, kt0 + 1):
                    vps = psot.tile([128, 64], sdt, tag="av", name="vps")
                    nc.tensor.transpose(
                        vps[:], KVT[b][0:64, kt * 128 : (kt + 1) * 128], ident[:]
                    )
                    nc.vector.tensor_copy(VO[b][:, kt, 0:64], vps[:])

            def emit_oproj_chunk(b, qs, nt, mh, act_cast=False):
                ns = qs + nt * 128
                ops = psot.tile([128, PSD], F32, tag="av", name="oprojps")
                nc.tensor.matmul(
                    ops[:], OT[b][:, ns // 128, :],
                    wo_sb[:, mh * PSD : (mh + 1) * PSD],
                )
                osb = outp.tile([128, PSD], sdt, tag="osb", name="oosb")
                # psum->bf16 cast: DVE normally; in the tail (exps done)
                # alternate onto the free ACT engine
                if act_cast:
                    nc.scalar.copy(osb[:], ops[:])
                else:
                    nc.vector.tensor_copy(osb[:], ops[:])
                # spread write traffic: b0 partials on sync, b1 on SWDGE
                eng = nc.sync if b == 0 else nc.gpsimd
                eng.dma_start(
                    out[b * N + ns : b * N + ns + 128,
                        mh * PSD : (mh + 1) * PSD],
                    osb[:],
                )

            # ---- attention emitters ----
            pts = {}      # (ti, kt, h) -> P tile awaiting AV
            o_ps = {}     # (ti, h) -> psum accumulator

            def emit_st_exp(ti, kt, h):
                b, qs = tiles[ti]
                st = stp.tile([128, QW], F32, tag=f"st{h}", name=f"st{h}")
                for h2 in range(2):
                    sl = slice(h2 * PSD, (h2 + 1) * PSD)
                    nc.tensor.matmul(
                        st[:, sl],
                        KT2[b][64 * h : 64 * h + 64, kt, :],
                        QT[b][64 * h : 64 * h + 64,
                              qs + h2 * PSD : qs + (h2 + 1) * PSD],
                    )
                pt = ptp.tile([128, QW], sdt, tag="pt", name="pt")
                nc.scalar.activation(pt[:], st[:], AF.Exp, scale=SCALE)
                pts[(ti, kt, h)] = pt

            def emit_av(ti, kt, h):
                b, qs = tiles[ti]
                if kt == 0 and h == 0:
                    # both accumulators allocated back-to-back: consecutive
                    # ring slots -> always distinct psum banks
                    o_ps[(ti, 0)] = psot.tile([65, QW], F32, tag="av",
                                              name="avac0")
                    o_ps[(ti, 1)] = psot.tile([65, QW], F32, tag="av",
                                              name="avac1")
                acc = o_ps[(ti, h)]
                pt = pts.pop((ti, kt, h))
                for h2 in range(2):
                    sl = slice(h2 * PSD, (h2 + 1) * PSD)
                    nc.tensor.matmul(
                        acc[:, sl], VO[b][:, kt, :], pt[:, sl],
                        start=(kt == 0), stop=(kt == KTS - 1),
                    )

            def emit_evac_norm(ti):
                """Evacuate AV accumulators (frees psot ring) + normalize.
                DVE + Pool only; nothing on ACT."""
                b, qs = tiles[ti]
                q0 = qs // 128
                osbs = []
                for h in range(2):
                    osb = stat.tile([65, QW], F32, tag=f"osb{h}", name=f"osb{h}")
                    # DVE evacuation (GPSIMD cannot read PSUM)
                    nc.vector.tensor_copy(osb[:], o_ps.pop((ti, h))[:])
                    osbs.append(osb)
                for h in range(2):
                    osb = osbs[h]
                    # custom DVE ops need base partition 0: stage sums row
                    ssb = stat.tile([1, QW], F32, tag="ssb", name="ssb", bufs=1)
                    nc.vector.tensor_copy(ssb[:], osb[64:65, :])
                    r = stat.tile([1, QW], F32, tag="r", name="r", bufs=1)
                    nc.vector.reciprocal_approx_fast(r[:], ssb[:])
                    rb = stat.tile([64, QW], F32, tag="rb", name="rb", bufs=1)
                    nc.gpsimd.partition_broadcast(rb[:], r[0:1, :])
                    # both muls on DVE: gpsimd must only ever run
                    # partition_broadcast, else its DSP library gets
                    # evicted and each boundary pays a ~7us lib reload
                    if h == 0:
                        nc.vector.tensor_mul(
                            OT[b][0:64, q0 : q0 + QW // 128, :],
                            osb[0:64, :].rearrange("p (k c) -> p k c", c=128),
                            rb[:].rearrange("p (k c) -> p k c", c=128),
                        )
                    else:
                        tmp = stat.tile([64, QW], sdt, tag="tmp", name="tmp",
                                        bufs=1)
                        nc.vector.tensor_mul(tmp[:], osb[0:64, :], rb[:])
                        nc.sync.dma_start(
                            OT[b][64:128, q0 : q0 + QW // 128, :],
                            tmp[:].rearrange("p (k c) -> p k c", c=128),
                        )

            # ---- fill regions: work interleaved into each tile's kt loop.
            # Region ti must fit that tile's PE slack (~10us = ~40 matmuls);
            # leftovers roll forward.  Each unit: (n_mms, emit_fn).
            regions = {ti: deque() for ti in range(NT + 1)}

            def region_add(ti, n_mms, fn):
                regions[ti].append((n_mms, fn))

            # region 0 (during b0/qt0): KVT(b0) 2nd half + QT(b0,qt1) + b1 x
            region_add(0, 0, lambda: emit_xt_load(1, 0))
            for half in range(2):
                region_add(0, 8, lambda h=half: emit_proj_chunk(0, QW, 1, h))
            region_add(0, 0, lambda: emit_kt2(0, 1))
            for kt0 in range(KTS // 2, KTS, 2):
                region_add(0, 2, lambda k=kt0: emit_transpose_pair(0, k))
            for half in range(2):
                region_add(0, 8, lambda h=half: emit_proj_chunk(0, QW, 0, h))
            region_add(0, 0, lambda: emit_xt_load(1, QW))

            # region 1 (during b0/qt1): b1 first half + QT(b1,qt0)
            for half in range(2):
                region_add(1, 8, lambda h=half: emit_proj_chunk(1, 0, 1, h))
            region_add(1, 0, lambda: emit_kt2(1, 0))
            for kt0 in range(0, KTS // 2, 2):
                region_add(1, 2, lambda k=kt0: emit_transpose_pair(1, k))
            for half in range(2):
                region_add(1, 8, lambda h=half: emit_proj_chunk(1, 0, 0, h))

            # region 2 (during b1/qt0): b1 second half + QT(b1,qt1)
            for half in range(2):
                region_add(2, 8, lambda h=half: emit_proj_chunk(1, QW, 1, h))
            region_add(2, 0, lambda: emit_kt2(1, 1))
            for kt0 in range(KTS // 2, KTS, 2):
                region_add(2, 2, lambda k=kt0: emit_transpose_pair(1, k))
            for half in range(2):
                region_add(2, 8, lambda h=half: emit_proj_chunk(1, QW, 0, h))

            # o_proj of tile ti can ride any window from ti+1 on (its OT is
            # ready just after the tile ti -> ti+1 boundary).  Budget each
            # chunk as ~3 mm: the DVE cast (~700ns), not the matmul, paces
            # an oproj-only stretch.  Tail chunks alternate the cast onto
            # ACT, which is idle once the exps are done.
            oproj_sched = [(0, 1, range(0, 8)), (0, 2, range(8, 16)),
                           (1, 3, range(0, 16)), (2, 3, range(0, 16)),
                           (3, 4, range(0, 16))]
            for ti, rgn, chunks in oproj_sched:
                b, qs = tiles[ti]
                for ch in chunks:
                    nt, mh = ch // 2, ch % 2
                    region_add(rgn, 3,
                               lambda b=b, q=qs, n=nt, m=mh, a=(rgn == 4):
                               emit_oproj_chunk(b, q, n, m, act_cast=a and
                                                (n * 2 + m) % 2 == 1))

            # ---- prologue: minimal work before the kt loop ----
            emit_xt_load(0, 0, split=True)
            emit_xt_load(0, QW)
            for half in range(2):
                emit_proj_chunk(0, 0, 1, half)      # KVT(b0, ns0)
            emit_kt2(0, 0)
            for kt0 in range(0, KTS // 2, 2):
                emit_transpose_pair(0, kt0)
            for half in range(2):
                emit_proj_chunk(0, 0, 0, half)      # QT(b0, qt0)

            # ---- main ACT-paced loop ----
            av_q = deque()            # pending (ti, kt, h) AV head-units

            def drain_one():
                emit_av(*av_q.popleft())

            for ti in range(NT):
                # psot discipline: fills may allocate psot only BEFORE this
                # tile's AV accumulators are allocated (fill window); once
                # the first AV is emitted, psot is owned by the accumulators
                # until the boundary evac.
                fill_window = True
                for kt in range(KTS):
                    if kt == 0:
                        # boundary: keep ACT fed first, then drain + evac
                        emit_st_exp(ti, 0, 0)
                        emit_st_exp(ti, 0, 1)
                        if ti > 0:
                            while av_q:
                                drain_one()
                            emit_evac_norm(ti - 1)
                        av_q.append((ti, 0, 0))
                        av_q.append((ti, 0, 1))
                        continue
                    emit_st_exp(ti, kt, 0)
                    emit_st_exp(ti, kt, 1)
                    if fill_window and regions[ti] \
                            and len(av_q) // 2 < CAP_KTS:
                        # AV paused: the whole ACT window minus ST is fill
                        budget = 1594.0
                        while budget > 0 and regions[ti] \
                                and len(av_q) // 2 < CAP_KTS:
                            n, fn = regions[ti].popleft()
                            fn()
                            budget -= max(n, 1) * 241.0
                    else:
                        fill_window = False
                        budget = SLOT_FILL_NS
                        emitted = False
                        while budget > 0 and len(av_q) > 2:
                            drain_one()
                            budget -= 482.0
                            emitted = True
                        if not emitted and FILLER:
                            for _ in range(FILLER):
                                nc.tensor.ldweights(ident[:, 0:1])
                    av_q.append((ti, kt, 0))
                    av_q.append((ti, kt, 1))
                regions[ti + 1].extendleft(reversed(regions[ti]))
                regions[ti].clear()

            # final boundary + tail
            while av_q:
                drain_one()
            emit_evac_norm(NT - 1)
            while regions[NT]:
                n, fn = regions[NT].popleft()
                fn()

            assert not pts and not o_ps

    nc.compile()
    return nc


def _get_nc(mode):
    key = (mode, P_DEPTH, SLOT_FILL_NS, FILLER)
    if key not in _NC_CACHE:
        _NC_CACHE[key] = _build_program(mode)
    return _NC_CACHE[key]


def _prep_in_maps(inputs, mode):
    ndt = _np_dt(mode)
    x = np.asarray(inputs["x"], np.float32)
    Wq = np.asarray(inputs["Wq"], np.float32)
    bq = np.asarray(inputs["bq"], np.float32)
    Wk = np.asarray(inputs["Wk"], np.float32)
    bk = np.asarray(inputs["bk"], np.float32)
    Wv = np.asarray(inputs["Wv"], np.float32)
    bv = np.asarray(inputs["bv"], np.float32)
    Wo = np.asarray(inputs["Wo"], np.float32)

    xT = np.ascontiguousarray(x.reshape(BN, D).T).astype(ndt)
    in_maps = []
    for i in range(NCORES):
        j0 = i * JC              # query-head column offset (heads 2i, 2i+1)
        g = i // 2               # kv head for this core
        v0 = g * HD
        wkv_i = np.concatenate(
            [Wv[:, v0 : v0 + HD], Wk[:, v0 : v0 + HD]], axis=1
        )  # V cols first (rows 0:64 of KVT), K cols second (rows 64:128)
        bkv_i = np.concatenate([bv[v0 : v0 + HD], bk[v0 : v0 + HD]])
        in_maps.append({
            "xT": xT,
            "wq": np.ascontiguousarray(Wq[:, j0 : j0 + JC]).astype(ndt),
            "wkv": np.ascontiguousarray(wkv_i).astype(ndt),
            "wo": np.ascontiguousarray(Wo[j0 : j0 + JC, :]).astype(ndt),
            "bq": np.ascontiguousarray(bq[j0 : j0 + JC]).reshape(JC, 1)
                    .astype(np.float32),
            "bkv": np.ascontiguousarray(bkv_i).reshape(JC, 1).astype(np.float32),
            "ident": np.eye(64, dtype=np.float32).astype(ndt),
            "ones": np.ones((128, KTS), dtype=np.float32).astype(ndt),
        })
    return in_maps


def _run(inputs, trace=False):
    mode = MM_MODE
    nc = _get_nc(mode)
    in_maps = _prep_in_maps(inputs, mode)
    res = run_bass_kernel_spmd(
        nc, in_maps, core_ids=list(range(NCORES)), trace=trace
    )
    bo = np.asarray(inputs["bo"], np.float32)
    acc = res.results[0]["out"].astype(np.float64)
    for i in range(1, NCORES):
        acc += res.results[i]["out"].astype(np.float64)
    full = (acc + bo.astype(np.float64)).astype(np.float32).reshape(B, N, D)
    return full, res


def kernel(**inputs):
    return _run(inputs, trace=False)[0]
